# revision 60
# baseline (speedup 1.0000x reference)
"""MoE gate (DeepSeek-V3 noaux_tc routing) on 8 Trainium2 NeuronCores.

Strategy: sequence-parallel — shard the 16384-token axis across 8 cores
(2048 tokens each), replicate the [256,7168] gate weight.

Numerics (validated vs fp64 in numpy; logits rms err ~1.8e-5, ~18 flipped
idx entries over the full batch -> metric ~8.5e-3, under the 2e-2 gate):
  logits = h1@w1                    fp16 x fp16   (1 col/cycle)
         + 2^-16 * [ h2@w1p + h1p@w2 ]            (one fp8 DoubleRow pass,
                                                   both terms packed in the
                                                   two per-cell slots)
  h1  = fp16(h)                     shipped from host (2 B/elem)
  h2  = fp8e4((h - h1) * 2^11)      shipped from host (1 B/elem)
  h1p = fp8e4(h1)                   shipped from host (1 B/elem)
  w1  = fp16(w);  w1p = fp8e4(w * 2^5);  w2 = fp8e4((w - w1) * 2^16)
Both DoubleRow slots accumulate at 2^16 scale, undone at PSUM readout.

Perf notes vs the previous 3-term bf16 version (CoreSim: 420.5us -> 155.4us,
PE busy 92%+ at its 143.4us floor):
  - PE work drops 768 -> 384 cols/chunk-tile (fp16 main + 0.5-rate fp8 DR).
  - All hidden-state DMAs use >=512B contiguous runs (no sub-512B penalty)
    via 256-token super-tiles and k-pair-interleaved fp8 host layouts.
  - DMA split across the three DGE queues (SP: h1, ACT: h2/h1p, Pool: w8 +
    s=0) with one-super-tile-ahead emission; the in-order queues otherwise
    let next-tile loads queue behind this tile's post-processing.
  - Prologue k-chunked so the first matmuls start at ~2.5us and are paced by
    the h1/w1 streams; group order per super-tile tuned so the fp8 pass
    never waits (s=0: mains first; last: j1-DR before j1-mains).
  - Outputs staged in SBUF, written in 2+2 batched DMAs at the end.
Top-k uses the DVE max8/max_index instructions (exact, stable-index);
per-group top-2 via eight [P,32] max8s; the score gather runs as 8
iota-match TensorScalarPtr ops on DVE (GPSIMD rejects that opcode).
"""
import sys
import os

sys.path.insert(0, "/opt/trn_rl_repo")

import numpy as np
import ml_dtypes

SEQ = 16384
HID = 7168
EXP = 256
N_CORES = 8
TOK = SEQ // N_CORES          # 2048 tokens per core
P = 128                       # partition dim / token tile
KT = HID // P                 # 56 contraction chunks
ST = 256                      # tokens per super-tile (DMA granularity)
NSUP = TOK // ST              # 8 super-tiles
SUBT = ST // P                # 2 sub-tiles (128 tokens) per super-tile
TILES = TOK // P              # 16 sub-tiles total
NG = 8                        # groups
GS = EXP // NG                # 32 experts per group
SCALE = 2.5
H2_SC = 2.0 ** 11             # h residual pre-scale (fp8 shipping)
W1P_SC = 2.0 ** 5             # w pre-scale for the fp8 copy
W2_SC = 2.0 ** 16             # w residual pre-scale
CORR_SC = 2.0 ** -16          # undo slot scales at PSUM readout

_CACHE = {}
LAST_RESULTS = None


def _build_program():
    import concourse.mybir as mybir
    import concourse.tile as tile
    from concourse import bacc

    nc = bacc.Bacc("TRN2", target_bir_lowering=False, debug=False,
                   num_devices=N_CORES)

    f16 = mybir.dt.float16
    f8 = mybir.dt.float8e4
    f32 = mybir.dt.float32
    DR = mybir.MatmulPerfMode.DoubleRow

    d_h1 = nc.dram_tensor("h1", [KT * P, NSUP * ST], f16, kind="ExternalInput").ap()
    d_h2 = nc.dram_tensor("h2", [KT // 2 * P, NSUP * 2 * ST], f8,
                          kind="ExternalInput").ap()
    # fp8(h1), host-prepared for every super-tile: an on-chip ACT cast was
    # tried first but its 12us/super-tile occupied ACT and stalled the first
    # DoubleRow groups; shipping it costs nothing (Pool DMA queue has slack)
    d_h1p = nc.dram_tensor("h1p", [KT // 2 * P, NSUP * 2 * ST], f8,
                           kind="ExternalInput").ap()
    d_w1 = nc.dram_tensor("w1", [KT * P, EXP], f16, kind="ExternalInput").ap()
    d_w8 = nc.dram_tensor("w8", [KT * P, 2 * EXP], f8, kind="ExternalInput").ap()
    d_bias = nc.dram_tensor("bias_rep", [P, EXP], f32, kind="ExternalInput").ap()
    d_iota = nc.dram_tensor("iota_rep", [P, EXP], f32, kind="ExternalInput").ap()
    d_ow = nc.dram_tensor("out_w", [TOK, 8], f32, kind="ExternalOutput").ap()
    d_oi = nc.dram_tensor("out_i", [TOK, 8], mybir.dt.int32, kind="ExternalOutput").ap()

    # DRAM access patterns (p = SBUF partition = contraction-within-chunk)
    h1_p = d_h1.rearrange("(k p) (s t) -> p k s t", p=P, t=ST)
    h2_p = d_h2.rearrange("(kk p) (s x) -> p kk s x", p=P, x=2 * ST)
    h1p_p = d_h1p.rearrange("(kk p) (s x) -> p kk s x", p=P, x=2 * ST)
    w1_p = d_w1.rearrange("(k p) e -> p k e", p=P)
    w8_p = d_w8.rearrange("(k p) x -> p k x", p=P)
    ow_p = d_ow.rearrange("(t p) k -> p t k", p=P)
    oi_p = d_oi.rearrange("(t p) k -> p t k", p=P)

    X = mybir.AxisListType.X
    op = mybir.AluOpType

    with tile.TileContext(nc) as tc:
        with tc.tile_pool(name="wpool", bufs=1) as wp, \
             tc.tile_pool(name="hpool", bufs=2) as hp, \
             tc.tile_pool(name="spool", bufs=2) as sp, \
             tc.tile_pool(name="smalls", bufs=2) as smp, \
             tc.tile_pool(name="opool", bufs=3) as outp, \
             tc.tile_pool(name="psum", bufs=2, space="PSUM") as pp, \
             tc.tile_pool(name="psum1", bufs=1, space="PSUM") as pp1:

            # --- persistent weights / bias / output staging ---
            NCH = 4
            KC = KT // NCH                 # 14 k-chunks per load chunk
            NCHF = 8
            KCF = KT // NCHF               # 7 k-chunks (finer, h1/w1 only)
            # first chunk extra-small so the very first matmul starts sooner
            CHB = [0, 3, 10, 17, 24, 32, 40, 48, 56]
            w1_t = wp.tile([P, KT * EXP], f16, tag="w1")
            w1_o = w1_t[:].rearrange("p (k e) -> p k e", e=EXP)
            w8_t = wp.tile([P, KT * 2 * EXP], f8, tag="w8")
            w8_o = w8_t[:].rearrange("p (k x) -> p k x", x=2 * EXP)
            for c in range(NCHF):
                ks = slice(CHB[c], CHB[c + 1])
                nc.scalar.dma_start(out=w1_o[:, ks, :], in_=w1_p[:, ks, :])
            stage_w = wp.tile([P, TILES * 8], f32, tag="stage_w")
            stage_i = wp.tile([P, TILES * 8], mybir.dt.uint32, tag="stage_i")

            w1_3 = w1_t[:].rearrange("p (k e) -> p k e", e=EXP)
            w8_3 = w8_t[:].rearrange("p (k sl e) -> p k sl e", sl=2, e=EXP)

            # Hidden-stream DMA emission runs one super-tile AHEAD of compute
            # (load(s+1) is emitted before chain(s)): the per-engine queues
            # are in-order, so next-tile loads must not sit behind this
            # tile's post-processing ops on the same queue.
            def load(s, chunked):
                h1_t = hp.tile([P, KT * ST], f16, tag="h1")
                h1_o = h1_t[:].rearrange("p (k t) -> p k t", t=ST)
                h8_t = hp.tile([P, 2 * KT * ST], f8, tag="h8")
                h8_slots = h8_t[:].rearrange("p (sl q) -> p sl q", sl=2)
                h2_o = h8_slots[:, 0, :].rearrange("p (kk x) -> p kk x",
                                                   x=2 * ST)
                h1p_o = h8_slots[:, 1, :].rearrange("p (kk x) -> p kk x",
                                                    x=2 * ST)
                if chunked:
                    # Prologue (~44us of DMA for w1/w8/h1/h2/h1p of s=0 plus
                    # s=1's loads) is queue-balanced and k-chunked; the k-wise
                    # dependencies let PE consume each chunk as it lands:
                    #   SP:   h1(0) chunks, h1p(0), h1(1) chunks
                    #   Pool: w8 + h2(0) chunks interleaved
                    #   ACT:  w1 chunks, then bias/iota + s>=1 h2/h1p streams
                    for c in range(NCHF):
                        ks = slice(CHB[c], CHB[c + 1])
                        nc.sync.dma_start(out=h1_o[:, ks, :],
                                          in_=h1_p[:, ks, s, :])
                    for c in range(NCH):
                        ks = slice(c * KC, (c + 1) * KC)
                        kks = slice(c * KC // 2, (c + 1) * KC // 2)
                        nc.gpsimd.dma_start(out=w8_o[:, ks, :],
                                            in_=w8_p[:, ks, :])
                        nc.gpsimd.dma_start(out=h2_o[:, kks, :],
                                            in_=h2_p[:, kks, s, :])
                        nc.sync.dma_start(out=h1p_o[:, kks, :],
                                          in_=h1p_p[:, kks, s, :])
                else:
                    # chunk h1 everywhere: the k-wise deps let the mains
                    # start on the first chunk if the SP queue is behind
                    for c in range(NCHF):
                        ks = slice(c * KCF, (c + 1) * KCF)
                        nc.sync.dma_start(out=h1_o[:, ks, :],
                                          in_=h1_p[:, ks, s, :])
                    nc.scalar.dma_start(out=h2_o, in_=h2_p[:, :, s, :])
                    nc.scalar.dma_start(out=h1p_o, in_=h1p_p[:, :, s, :])
                return h1_t, h8_t

            pending = load(0, chunked=True)
            # bias/iota after the streaming prologue DMAs on ACT (needed only
            # by the first post-processing chain, ~19us in)
            bias_t = wp.tile([P, EXP], f32, tag="bias")
            nc.scalar.dma_start(out=bias_t[:], in_=d_bias)
            iota_t = wp.tile([P, EXP], f32, tag="iota")
            nc.scalar.dma_start(out=iota_t[:], in_=d_iota)

            for s in range(NSUP):
                h1_t, h8_t = pending
                if s + 1 < NSUP:
                    pending = load(s + 1, chunked=False)

                h1_3 = h1_t[:].rearrange("p (k t) -> p k t", t=ST)
                # [p][slot][kk][x=par*ST+t] view for DoubleRow stationaries
                h8_4 = h8_t[:].rearrange("p (sl kk x) -> p sl kk x",
                                         sl=2, x=2 * ST)

                def mains(j):
                    ts = slice(j * P, (j + 1) * P)
                    ps_m = pp.tile([P, EXP], f32, tag=f"ps_m{j}")
                    for k in range(KT):
                        nc.tensor.matmul(
                            ps_m[:, :], h1_3[:, k, ts], w1_3[:, k, :],
                            start=(k == 0), stop=(k == KT - 1),
                            skip_group_check=True)
                    return ps_m

                def corr(j):
                    # one fp8 DoubleRow pass; slot0=h2@w1p, slot1=h1p@w2
                    ps_c = pp.tile([P, EXP], f32, tag="ps_c")
                    for k in range(KT):
                        kk, par = divmod(k, 2)
                        xs = slice(par * ST + j * P, par * ST + (j + 1) * P)
                        nc.tensor.matmul(
                            ps_c[:, :], h8_4[:, :, kk, xs], w8_3[:, k, :, :],
                            start=(k == 0), stop=(k == KT - 1),
                            perf_mode=DR, skip_group_check=True)
                    return ps_c

                def mains_and_corr(j):
                    # k-interleaved variant: sim-neutral (ldweights isn't
                    # modeled) but on hardware each DoubleRow 256-col weight
                    # load hides under the neighboring 256-cycle fp16 matmul
                    ts = slice(j * P, (j + 1) * P)
                    ps_m = pp.tile([P, EXP], f32, tag=f"ps_m{j}")
                    ps_c = pp.tile([P, EXP], f32, tag="ps_c")
                    for k in range(KT):
                        kk, par = divmod(k, 2)
                        xs = slice(par * ST + j * P, par * ST + (j + 1) * P)
                        nc.tensor.matmul(
                            ps_m[:, :], h1_3[:, k, ts], w1_3[:, k, :],
                            start=(k == 0), stop=(k == KT - 1),
                            skip_group_check=True)
                        nc.tensor.matmul(
                            ps_c[:, :], h8_4[:, :, kk, xs], w8_3[:, k, :, :],
                            start=(k == 0), stop=(k == KT - 1),
                            perf_mode=DR, skip_group_check=True)
                    return ps_m, ps_c

                def chain(j, ps_m_parts, ps_c):
                    # ps_m_parts: list of (expert col slice, psum tile) for
                    # the main term; >1 part lets the per-half front of the
                    # chain overlap the remaining main matmuls (last tile)
                    t_idx = s * SUBT + j
                    # --- logits = main + 2^-16 * corr; scores = sigmoid ---
                    xterm = sp.tile([P, EXP], f32, tag="xterm")
                    nc.scalar.copy(xterm[:], ps_c[:, :])
                    logits = sp.tile([P, EXP], f32, tag="logits")
                    scores = sp.tile([P, EXP], f32, tag="scores")
                    s4c = sp.tile([P, EXP], f32, tag="s4c")
                    for es, ps_m in ps_m_parts:
                        nc.vector.scalar_tensor_tensor(
                            out=logits[:, es], in0=xterm[:, es], scalar=CORR_SC,
                            in1=ps_m[:, :], op0=op.mult, op1=op.add)
                        nc.scalar.activation(scores[:, es], logits[:, es],
                                             mybir.ActivationFunctionType.Sigmoid)
                        nc.vector.tensor_add(s4c[:, es], scores[:, es],
                                             bias_t[:, es])
                    s4c3 = s4c[:].rearrange("p (g e) -> p g e", e=GS)

                    # --- group scores: per-group max8 -> top1+top2 sum ---
                    gtop = smp.tile([P, NG * 8], f32, tag="gtop")
                    for g in range(NG):
                        nc.vector.max(out=gtop[:, g * 8:(g + 1) * 8],
                                      in_=s4c[:, g * GS:(g + 1) * GS])
                    gsum = smp.tile([P, NG], f32, tag="gsum")
                    nc.vector.reduce_sum(
                        gsum[:],
                        gtop[:].rearrange("p (g x) -> p g x", x=8)[:, :, 0:2],
                        axis=X)

                    # --- top-4 groups -> expert mask -> masked scores ---
                    gsort = smp.tile([P, 8], f32, tag="gsort")
                    nc.vector.max(out=gsort[:], in_=gsum[:])
                    gmask = smp.tile([P, NG], f32, tag="gmask")
                    nc.vector.tensor_scalar(
                        out=gmask[:], in0=gsum[:], scalar1=gsort[:, 3:4],
                        scalar2=None, op0=op.is_ge)
                    tmp = sp.tile([P, EXP], f32, tag="tmp")
                    nc.vector.tensor_tensor(
                        tmp[:].rearrange("p (g e) -> p g e", e=GS), s4c3,
                        gmask[:].to_broadcast([P, NG, GS]), op=op.mult)

                    # --- top-8 over masked corrected scores ---
                    v8 = outp.tile([P, 8], f32, tag="v8")
                    nc.vector.max(out=v8[:], in_=tmp[:])
                    idx8 = stage_i[:, t_idx * 8:(t_idx + 1) * 8]
                    nc.vector.max_index(out=idx8, in_max=v8[:], in_values=tmp[:])

                    # --- gather uncorrected scores at the top-8 positions ---
                    # keyed on the (unique) index, not the value: two experts
                    # can have bitwise-equal corrected scores
                    idxf = outp.tile([P, 8], f32, tag="idxf")
                    nc.vector.tensor_copy(idxf[:], idx8)
                    wsel = outp.tile([P, 8], f32, tag="wsel")
                    scratch = sp.tile([P, EXP], f32, tag="scratch")
                    for kk in range(8):
                        # (must stay on DVE: walrus rejects TensorScalarPtr
                        # on the Pool/GPSIMD engine)
                        nc.vector.scalar_tensor_tensor(
                            out=scratch[:], in0=iota_t[:],
                            scalar=idxf[:, kk:kk + 1], in1=scores[:],
                            op0=op.is_equal, op1=op.mult,
                            accum_out=wsel[:, kk:kk + 1])

                    # --- renormalize * 2.5 ---
                    denom = smp.tile([P, 1], f32, tag="denom")
                    nc.vector.reduce_sum(denom[:], wsel[:], axis=X)
                    nc.vector.tensor_scalar_add(denom[:], denom[:], 1e-20)
                    recip = smp.tile([P, 1], f32, tag="recip")
                    nc.vector.reciprocal(recip[:], denom[:])
                    nc.vector.tensor_scalar(
                        out=stage_w[:, t_idx * 8:(t_idx + 1) * 8], in0=wsel[:],
                        scalar1=recip[:, 0:1], scalar2=SCALE,
                        op0=op.mult, op1=op.mult)

                if s == 0:
                    # prologue DMAs are still streaming in: run both main
                    # groups first so the DR groups start once w8/h2/h1p land
                    ps_m0, ps_m1 = mains(0), mains(1)
                    chain(0, [(slice(0, EXP), ps_m0)], corr(0))
                    chain(1, [(slice(0, EXP), ps_m1)], corr(1))
                elif s == NSUP - 1:
                    # last super-tile: j=1 DR group runs before the j=1 mains
                    # (the PSUM-corr bounce overlaps them), and the j=1 mains
                    # are split by expert half so the front of the final chain
                    # (logits/sigmoid/s4c/group-maxes of half 1) overlaps the
                    # second half's matmuls instead of trailing the kernel
                    ps_m0 = mains(0)
                    chain(0, [(slice(0, EXP), ps_m0)], corr(0))
                    ps_c1 = corr(1)
                    ts1 = slice(P, 2 * P)
                    parts = []
                    for h, es in enumerate((slice(0, 160), slice(160, EXP))):
                        ncols = es.stop - es.start
                        ps_h = pp1.tile([P, ncols], f32, tag=f"ps_m1h{h}")
                        for k in range(KT):
                            nc.tensor.matmul(
                                ps_h[:, :], h1_3[:, k, ts1], w1_3[:, k, es],
                                start=(k == 0), stop=(k == KT - 1),
                                skip_group_check=True)
                        parts.append((es, ps_h))
                    chain(1, parts, ps_c1)
                else:
                    # interleave so each tile's DVE chain starts early
                    for j in range(SUBT):
                        ps_m, ps_c = mains_and_corr(j)
                        chain(j, [(slice(0, EXP), ps_m)], ps_c)

            # --- batched output DMAs: bulk after tile 13, remainder at end ---
            sw3 = stage_w[:].rearrange("p (t k) -> p t k", k=8)
            si3 = stage_i[:].rearrange("p (t k) -> p t k", k=8).bitcast(
                mybir.dt.int32)
            nc.sync.dma_start(out=ow_p[:, 0:TILES - 2, :],
                              in_=sw3[:, 0:TILES - 2, :])
            nc.sync.dma_start(out=oi_p[:, 0:TILES - 2, :],
                              in_=si3[:, 0:TILES - 2, :])
            # final two tiles go per-tile; tile 14 on ACT overlaps tile 15's
            # chain, and tile 15's two outputs go to different queues so the
            # closing writes run in parallel
            t = TILES - 2
            nc.scalar.dma_start(out=ow_p[:, t:t + 1, :], in_=sw3[:, t:t + 1, :])
            nc.scalar.dma_start(out=oi_p[:, t:t + 1, :], in_=si3[:, t:t + 1, :])
            t = TILES - 1
            nc.scalar.dma_start(out=ow_p[:, t:t + 1, :], in_=sw3[:, t:t + 1, :])
            nc.gpsimd.dma_start(out=oi_p[:, t:t + 1, :], in_=si3[:, t:t + 1, :])

    nc.compile()
    return nc


def _get_program():
    if "nc" not in _CACHE:
        _CACHE["nc"] = _build_program()
    return _CACHE["nc"]


def _prepare_in_maps(hidden_states, weight, e_score_correction_bias):
    h = np.asarray(hidden_states, dtype=np.float32)
    w = np.asarray(weight, dtype=np.float32)
    b = np.asarray(e_score_correction_bias, dtype=np.float32)

    f16 = np.float16
    f8 = ml_dtypes.float8_e4m3

    wT = np.ascontiguousarray(w.T)                      # [HID, EXP]
    w1 = wT.astype(f16)
    w1p = (wT * W1P_SC).astype(f8)
    w2 = ((wT - w1.astype(np.float32)) * W2_SC).astype(f8)
    # stack axis=1 over HID gives [HID, 2, EXP]; flatten keeps (k p) major
    w8 = np.ascontiguousarray(
        np.stack([w1p, w2], axis=1)).reshape(KT * P, 2 * EXP)
    w1 = np.ascontiguousarray(w1)

    bias_rep = np.ascontiguousarray(np.broadcast_to(b[None, :], (P, EXP)))
    iota_rep = np.ascontiguousarray(
        np.broadcast_to(np.arange(EXP, dtype=np.float32)[None, :], (P, EXP)))

    hT = h.T                                            # [HID, SEQ] (view)
    h1_full = np.ascontiguousarray(hT).astype(f16)      # [HID, SEQ]
    resid = (hT - h1_full.astype(np.float32)) * H2_SC
    h2_full = resid.astype(f8)                          # [HID, SEQ]

    in_maps = []
    for c in range(N_CORES):
        sl = slice(c * TOK, (c + 1) * TOK)
        h1c = h1_full[:, sl]                            # [HID, TOK]
        # [HID, TOK] -> [(k p), (s t)] is the natural reshape
        h1c = np.ascontiguousarray(h1c).reshape(KT * P, TOK)
        h2c = h2_full[:, sl]                            # [HID, TOK]
        # -> [kk, par, p, s, t] -> [kk, p, s, par, t] -> [(kk p), (s par t)]
        h2c = np.ascontiguousarray(
            h2c.reshape(KT // 2, 2, P, NSUP, ST).transpose(0, 2, 3, 1, 4)
        ).reshape(KT // 2 * P, NSUP * 2 * ST)
        # fp8(h1) in h8-slot1 layout, same dim order as h2
        h1pc = np.ascontiguousarray(
            h1_full[:, sl].astype(f8)
            .reshape(KT // 2, 2, P, NSUP, ST).transpose(0, 2, 3, 1, 4)
        ).reshape(KT // 2 * P, NSUP * 2 * ST)
        in_maps.append({
            "h1": h1c,
            "h2": h2c,
            "h1p": h1pc,
            "w1": w1,
            "w8": w8,
            "bias_rep": bias_rep,
            "iota_rep": iota_rep,
        })
    return in_maps


def kernel(hidden_states, weight, e_score_correction_bias):
    global LAST_RESULTS
    from concourse.bass_utils import run_bass_kernel_spmd

    nc = _get_program()
    in_maps = _prepare_in_maps(hidden_states, weight, e_score_correction_bias)

    trace = bool(int(os.environ.get("KERNEL_TRACE", "0")))
    res = run_bass_kernel_spmd(nc, in_maps, core_ids=list(range(N_CORES)),
                               trace=trace)
    LAST_RESULTS = res

    topk_w = np.concatenate([res.results[c]["out_w"] for c in range(N_CORES)], axis=0)
    topk_i = np.concatenate([res.results[c]["out_i"] for c in range(N_CORES)], axis=0)
    return topk_w, topk_i
